# revision 11
# baseline (speedup 1.0000x reference)
"""GQA causal attention block (x @ Wq/Wk/Wv -> causal GQA attention -> @ Wo)
for Trainium2, SPMD over 8 NeuronCores.

Sharding: 4 batches x 2 query-shards. Core c handles batch c//2 and the
interleaved set of 128-row query tiles {s, s+2, s+4, ...} (s = c%2), which
balances the causal-attention triangle between the two shards of a batch.
Each core computes k/v projections for the full sequence (cheap), q/o
projections and attention only for its own query rows, and writes its own
output rows -- no collectives. The host scatters the per-core outputs back
into the full tensor.

The SPMD program is identical on all cores; per-shard differences (which
query rows, causal mask alignment) are carried entirely by the input data
(a gathered "xq" tensor and two host-provided mask tiles).
"""

import sys

for _p in ("/opt/trn_rl_repo", "/root/.axon_site/_ro/trn_rl_repo"):
    if _p not in sys.path:
        sys.path.append(_p)

import numpy as np
import ml_dtypes

import concourse.bacc as bacc
import concourse.tile as tile
import concourse.mybir as mybir
from concourse.bass_utils import run_bass_kernel_spmd

F32 = mybir.dt.float32
F32R = mybir.dt.float32r
BF16 = mybir.dt.bfloat16
AF = mybir.ActivationFunctionType
NEG = -1.0e6  # additive mask for disallowed keys (pre-softmax-scale)


class Cfg:
    def __init__(self, T, E, H, KV, n_batch, n_shard, blk):
        self.T, self.E, self.H, self.KV = T, E, H, KV
        self.D = 128
        self.G = H // KV
        self.NE = E // 128           # contraction chunks for projections
        self.n_batch = n_batch
        self.n_shard = n_shard
        self.n_cores = n_batch * n_shard
        self.RQ = T // n_shard       # query rows per core
        self.NJ = self.RQ // 128     # local 128-row query tiles
        self.BLK = blk               # query block for q-proj/o-proj
        self.NB = self.RQ // blk
        self.JPB = blk // 128
        self.NTA = T // 512          # phase-A 512-row t-tiles
        self.HKV = KV * self.D       # k/v projection width
        self.scale = 1.0 / float(np.sqrt(self.D))


FULL = Cfg(T=2048, E=2048, H=16, KV=4, n_batch=4, n_shard=2, blk=512)


def build(cfg):
    c = cfg
    nc = bacc.Bacc("TRN2", target_bir_lowering=False, debug=False,
                   num_devices=c.n_cores)

    x_d = nc.dram_tensor("x", [c.T, c.E], F32, kind="ExternalInput").ap()
    xq_d = nc.dram_tensor("xq", [c.RQ, c.E], F32, kind="ExternalInput").ap()
    wq_d = nc.dram_tensor("Wq", [c.E, c.H * c.D], F32R, kind="ExternalInput").ap()
    wk_d = nc.dram_tensor("Wk", [c.E, c.HKV], F32R, kind="ExternalInput").ap()
    wv_d = nc.dram_tensor("Wv", [c.E, c.HKV], F32R, kind="ExternalInput").ap()
    wo_d = nc.dram_tensor("Wo", [c.H * c.D, c.E], F32R, kind="ExternalInput").ap()
    mask_d = nc.dram_tensor("masks", [c.n_shard, 128, 128], F32,
                            kind="ExternalInput").ap()
    idb_d = nc.dram_tensor("identb", [128, 128], BF16, kind="ExternalInput").ap()
    idf_d = nc.dram_tensor("identf", [128, 128], F32, kind="ExternalInput").ap()
    o_d = nc.dram_tensor("o", [c.RQ, c.E], F32, kind="ExternalOutput").ap()

    with tile.TileContext(nc) as tc:
        with (
            tc.tile_pool(name="const", bufs=1) as constp,
            tc.tile_pool(name="kv", bufs=c.KV) as kvp,
            tc.tile_pool(name="vsb", bufs=c.T // 128) as vp,
            tc.tile_pool(name="qt", bufs=c.H) as qtp,
            tc.tile_pool(name="yt", bufs=c.H) as ytp,
            tc.tile_pool(name="xn", bufs=2) as xnp,
            tc.tile_pool(name="xta", bufs=c.NE + 1) as xtap,
            tc.tile_pool(name="wt", bufs=2) as wtp,
            tc.tile_pool(name="pr", bufs=2) as prp,
            tc.tile_pool(name="sm", bufs=6) as smp,
            tc.tile_pool(name="ps5", bufs=5, space="PSUM") as ps5,
            tc.tile_pool(name="ps2", bufs=2, space="PSUM") as ps2,
            tc.tile_pool(name="ps1", bufs=1, space="PSUM") as ps1,
        ):
            # --- constants ---
            masks = []
            for i in range(c.n_shard):
                m = constp.tile([128, 128], F32, tag=f"mask{i}")
                nc.sync.dma_start(m[:], mask_d[i])
                masks.append(m)
            identb = constp.tile([128, 128], BF16, tag="identb")
            nc.sync.dma_start(identb[:], idb_d[:])
            identf = constp.tile([128, 128], F32, tag="identf")
            nc.sync.dma_start(identf[:], idf_d[:])

            # persistent activations
            kT = [kvp.tile([128, c.T], F32R, tag="kT", name=f"kT{i}") for i in range(c.KV)]
            v_sb = [vp.tile([128, c.HKV], BF16, tag="v", name=f"v{i}") for i in range(c.T // 128)]

            # ---------------- Phase A: k/v projection over full T -----------
            for tt in range(c.NTA):
                # transpose this 512-row t-tile into NE [128, 512] xT tiles
                xts = [xtap.tile([128, 512], F32R, tag="xta", name=f"xta{e}")
                       for e in range(c.NE)]
                for i in range(4):
                    for half in range(2):
                        xn = xnp.tile([128, c.E // 2], F32, tag="xn")
                        nc.sync.dma_start(
                            xn[:], x_d[tt * 512 + i * 128:
                                       tt * 512 + (i + 1) * 128,
                                       half * (c.E // 2):(half + 1) * (c.E // 2)])
                        for eh in range(c.NE // 2):
                            e = half * (c.NE // 2) + eh
                            pt = ps2.tile([128, 128], F32, tag="tp")
                            nc.tensor.transpose(
                                pt[:], xn[:, eh * 128:(eh + 1) * 128], identf[:])
                            nc.vector.tensor_copy(
                                xts[e][:, i * 128:(i + 1) * 128], pt[:])
                # kT pass
                psk = [ps5.tile([128, 512], F32, tag="ps512", name=f"psk{i}")
                       for i in range(c.KV)]
                for e in range(c.NE):
                    wk_t = wtp.tile([128, c.HKV], F32R, tag="wk")
                    nc.sync.dma_start(wk_t[:], wk_d[e * 128:(e + 1) * 128, :])
                    for h in range(c.KV):
                        nc.tensor.matmul(psk[h][:],
                                         wk_t[:, h * 128:(h + 1) * 128],
                                         xts[e][:],
                                         start=(e == 0), stop=(e == c.NE - 1))
                for h in range(c.KV):
                    nc.vector.tensor_copy(kT[h][:, tt * 512:(tt + 1) * 512], psk[h][:])
                # v pass
                psv = [ps5.tile([128, c.HKV], F32, tag="ps512", name=f"psv{i}")
                       for i in range(4)]
                for e in range(c.NE):
                    wv_t = wtp.tile([128, c.HKV], F32R, tag="wv")
                    nc.sync.dma_start(wv_t[:], wv_d[e * 128:(e + 1) * 128, :])
                    for i in range(4):
                        nc.tensor.matmul(psv[i][:],
                                         xts[e][:, i * 128:(i + 1) * 128],
                                         wv_t[:],
                                         start=(e == 0), stop=(e == c.NE - 1))
                for i in range(4):
                    nc.scalar.copy(v_sb[tt * 4 + i][:], psv[i][:])

            # ---------------- Phase B: per query block ----------------------
            for blk in range(c.NB):
                # transpose xq block
                xqt = [xtap.tile([128, c.BLK], F32R, tag="xta", name=f"xta{e}")
                       for e in range(c.NE)]
                for i in range(c.JPB):
                    for half in range(2):
                        xn = xnp.tile([128, c.E // 2], F32, tag="xn")
                        nc.sync.dma_start(
                            xn[:], xq_d[blk * c.BLK + i * 128:
                                        blk * c.BLK + (i + 1) * 128,
                                        half * (c.E // 2):(half + 1) * (c.E // 2)])
                        for eh in range(c.NE // 2):
                            e = half * (c.NE // 2) + eh
                            pt = ps2.tile([128, 128], F32, tag="tp")
                            nc.tensor.transpose(
                                pt[:], xn[:, eh * 128:(eh + 1) * 128], identf[:])
                            nc.vector.tensor_copy(
                                xqt[e][:, i * 128:(i + 1) * 128], pt[:])

                # q projection, 4 heads at a time
                qT = []
                for hg in range(c.H // 4):
                    psq = [ps5.tile([128, c.BLK], F32, tag="ps512", name=f"psq{i}")
                           for i in range(4)]
                    for e in range(c.NE):
                        wq_t = wtp.tile([128, 512], F32R, tag="wq")
                        nc.sync.dma_start(
                            wq_t[:], wq_d[e * 128:(e + 1) * 128,
                                          hg * 512:(hg + 1) * 512])
                        for hh in range(4):
                            nc.tensor.matmul(
                                psq[hh][:],
                                wq_t[:, hh * 128:(hh + 1) * 128],
                                xqt[e][:],
                                start=(e == 0), stop=(e == c.NE - 1))
                    for hh in range(4):
                        q = qtp.tile([128, c.BLK], F32R, tag="qT")
                        nc.vector.tensor_copy(q[:], psq[hh][:])
                        qT.append(q)

                # attention per local q-tile and head
                yT = [ytp.tile([128, c.BLK], F32R, tag="yT", name=f"yT{i}") for i in range(c.H)]
                for jj in range(c.JPB):
                    j = blk * c.JPB + jj
                    nk = c.n_shard * (j + 1)      # 128-key subtiles incl. masked
                    nch = (nk * 128 + 511) // 512
                    for h in range(c.H):
                        kv = h // c.G
                        probs = prp.tile([128, c.NJ * c.n_shard * 128], BF16,
                                         tag="probs")
                        sums = smp.tile([128, 8], F32, tag="sums")
                        for ch in range(nch):
                            cs = ch * 512
                            w = min(512, nk * 128 - cs)
                            pss = ps5.tile([128, w], F32, tag="ps512")
                            nc.tensor.matmul(
                                pss[:],
                                qT[h][:, jj * 128:(jj + 1) * 128],
                                kT[kv][:, cs:cs + w],
                                start=True, stop=True)
                            for mi in range(c.n_shard):
                                kk = nk - c.n_shard + mi
                                if cs <= kk * 128 < cs + w:
                                    off = kk * 128 - cs
                                    nc.vector.tensor_add(
                                        pss[:, off:off + 128],
                                        pss[:, off:off + 128], masks[mi][:])
                            nc.scalar.activation(
                                probs[:, cs:cs + w], pss[:], AF.Exp,
                                scale=c.scale,
                                accum_out=sums[:, ch:ch + 1])
                        tot = smp.tile([128, 1], F32, tag="tot")
                        nc.vector.reduce_sum(tot[:], sums[:, :nch],
                                             axis=mybir.AxisListType.X)
                        recip = smp.tile([128, 1], F32, tag="recip")
                        nc.vector.reciprocal(recip[:], tot[:])
                        psy = ps1.tile([128, 128], F32, tag="yt")
                        for kk in range(nk):
                            pb = smp.tile([128, 128], BF16, tag="pb")
                            nc.scalar.activation(pb[:],
                                                 probs[:, kk * 128:(kk + 1) * 128],
                                                 AF.Copy, scale=recip[:])
                            ptr = ps2.tile([128, 128], BF16, tag="tp")
                            nc.tensor.transpose(ptr[:], pb[:], identb[:])
                            pT = smp.tile([128, 128], BF16, tag="pT")
                            nc.any.tensor_copy(pT[:], ptr[:])
                            nc.tensor.matmul(psy[:],
                                             v_sb[kk][:, kv * 128:(kv + 1) * 128],
                                             pT[:],
                                             start=(kk == 0), stop=(kk == nk - 1))
                        nc.vector.tensor_copy(yT[h][:, jj * 128:(jj + 1) * 128],
                                              psy[:])

                # o projection for this block
                for et in range(c.E // 512):
                    pso = [ps5.tile([128, 512], F32, tag="ps512", name=f"pso{i}")
                           for i in range(c.JPB)]
                    for h in range(c.H):
                        wo_t = wtp.tile([128, 512], F32R, tag="wo")
                        nc.sync.dma_start(
                            wo_t[:], wo_d[h * 128:(h + 1) * 128,
                                          et * 512:(et + 1) * 512])
                        for tsub in range(c.JPB):
                            nc.tensor.matmul(
                                pso[tsub][:],
                                yT[h][:, tsub * 128:(tsub + 1) * 128],
                                wo_t[:],
                                start=(h == 0), stop=(h == c.H - 1))
                    for tsub in range(c.JPB):
                        r0 = (blk * c.JPB + tsub) * 128
                        osb = wtp.tile([128, 512], F32, tag="osb")
                        nc.any.tensor_copy(osb[:], pso[tsub][:])
                        nc.sync.dma_start(o_d[r0:r0 + 128, et * 512:(et + 1) * 512],
                                          osb[:])

    nc.compile()
    return nc


def make_masks(cfg, s):
    """Additive masks for the last n_shard key subtiles of each query row.

    For shard s, local q-tile j maps to global tile g = j*n_shard + s; the
    program processes key subtiles [0, j*n_shard + n_shard) and adds masks[mi]
    to key subtile j*n_shard + mi.  mi < s: fully allowed; mi == s: causal
    triangle; mi > s: fully masked.
    """
    r = np.arange(128)
    tri = np.where(r[:, None] >= r[None, :], 0.0, NEG).astype(np.float32)
    out = np.zeros((cfg.n_shard, 128, 128), np.float32)
    for mi in range(cfg.n_shard):
        if mi == s:
            out[mi] = tri
        elif mi > s:
            out[mi] = NEG
    return out


def make_inputs(cfg, x, Wq, Wk, Wv, Wo):
    """Per-core input maps from full tensors."""
    ident_f = np.eye(128, dtype=np.float32)
    ident_b = np.eye(128, dtype=ml_dtypes.bfloat16)
    in_maps = []
    for c in range(cfg.n_cores):
        b, s = divmod(c, cfg.n_shard)
        xb = np.ascontiguousarray(x[b])
        xq = np.ascontiguousarray(
            xb.reshape(cfg.T // 128, 128, cfg.E)[s::cfg.n_shard]
            .reshape(cfg.RQ, cfg.E))
        in_maps.append({
            "x": xb, "xq": xq, "Wq": Wq, "Wk": Wk, "Wv": Wv, "Wo": Wo,
            "masks": make_masks(cfg, s),
            "identb": ident_b, "identf": ident_f,
        })
    return in_maps


def scatter_out(cfg, results):
    B = cfg.n_batch
    out = np.empty((B, cfg.T, cfg.E), np.float32)
    for c in range(cfg.n_cores):
        b, s = divmod(c, cfg.n_shard)
        out[b].reshape(cfg.T // 128, 128, cfg.E)[s::cfg.n_shard] = \
            results[c]["o"].reshape(cfg.RQ // 128, 128, cfg.E)
    return out


_NC_CACHE = {}


def get_nc(cfg):
    key = (cfg.T, cfg.E, cfg.H, cfg.KV, cfg.n_batch, cfg.n_shard, cfg.BLK)
    if key not in _NC_CACHE:
        _NC_CACHE[key] = build(cfg)
    return _NC_CACHE[key]


def run_on_hw(cfg, x, Wq, Wk, Wv, Wo, trace=False):
    nc = get_nc(cfg)
    in_maps = make_inputs(cfg, x, Wq, Wk, Wv, Wo)
    res = run_bass_kernel_spmd(nc, in_maps, list(range(cfg.n_cores)),
                               trace=trace)
    return scatter_out(cfg, [r for r in res.results]), res


def kernel(x, Wq, Wk, Wv, Wo):
    out, _ = run_on_hw(FULL, np.asarray(x), np.asarray(Wq), np.asarray(Wk),
                       np.asarray(Wv), np.asarray(Wo))
    return out
